# revision 12
# baseline (speedup 1.0000x reference)
"""Trainium2 Bass kernel for causal multi-head attention with interleaved RoPE.

Problem: B=2, S=2048, D=1024, 16 heads x 64 dims, causal, rope theta=1e4.

Sharding (8 cores): 2-way batch x 4-way head tensor-parallel.
  core i: batch b = i // 4, head group g = i % 4 (heads 4g..4g+3, dims 256).
  Each core computes q/k/v for its heads from x[b], runs causal flash
  attention, and produces a partial output projection outT = wo_g.T-slice
  contribution [D, S].  Host sums the 4 partials per batch and transposes.

v2 layout notes:
  - All matmul operands are bf16 (PSUM accumulation stays f32); tolerance is
    2e-2 so bf16's ~0.2% error is far inside budget.  bf16 streams at 1
    cycle/row on the PE for any N and halves SBUF/DMA traffic.
  - qT/kT are computed transposed ([dim, row]) via lhsT=weights, rhs=x^T.
  - RoPE pairs are de-interleaved on the host by permuting weight columns so
    pair partners sit 16 partitions apart (within a 32-partition quadrant),
    making the partner fetch a single DVE stream_shuffle.
  - Scores are computed transposed (S^T[k, q]) so the AV matmul needs no
    transposes; the softmax normalizer comes from a ones-row appended to V.
  - Causality: only k-chunks up to the diagonal are processed; diagonal-band
    128x128 blocks get a -60 triangular mask add before exp.
  - Attention is software-pipelined: the AV matmul for chunk kc is emitted
    after the score matmuls for chunk kc+1, so the Tensor engine never waits
    on the Scalar-engine exp.
  - Normalization is per-(qt,g) and stays on-chip: sums row -> K=1 broadcast
    matmul -> reciprocal_approx_fast -> fused multiply into bf16 oT.
"""

import os
import sys

sys.path.insert(0, "/opt/trn_rl_repo")

import numpy as np
import ml_dtypes

B = 2
S = 2048
D = 1024
NH = 16
HD = 64
THETA = 10000.0
NCORES = 8
HPC = 4  # heads per core
DC = HPC * HD  # 256 dims per core
GQ = 2  # 128-partition groups per core for q/k/o dims (DC/128)
QT = 512  # query tile (free dim)
NQT = S // QT
KC = 128  # key chunk (partition dim)
NKC = S // KC
MASKVAL = -60.0
BF16 = ml_dtypes.bfloat16

_CACHE = {}


def _install_axon_ntff_hook():
    """Register antenv.axon_hooks so trace=True (BASS_TRACE=1) works."""
    import types

    if "antenv.axon_hooks" in sys.modules:
        return
    m = types.ModuleType("antenv.axon_hooks")
    _hook = [None]
    m.set_axon_ntff_profile_hook = lambda h: _hook.__setitem__(0, h)
    m.get_axon_ntff_profile_hook = lambda: _hook[0]
    sys.modules["antenv.axon_hooks"] = m
    try:
        import antenv

        antenv.axon_hooks = m
        from trn_agent_boot.trn_boot import _ntff_profile_via_ctypes

        hook = _ntff_profile_via_ctypes("/opt/axon/libaxon_pjrt.so")
        if hook is not None:
            m.set_axon_ntff_profile_hook(hook)
    except Exception:
        pass


def _rope_perm_local():
    """Permutation of one head's 64 dims: original interleaved pair (2i, 2i+1)
    -> t0 at quadrant*32 + (i%16), t1 at quadrant*32 + 16 + (i%16), with
    quadrant = i // 16.  Returns perm such that new[j] = old[perm[j]]."""
    perm = np.zeros(HD, dtype=np.int64)
    for i in range(HD // 2):
        qd, r = divmod(i, 16)
        perm[qd * 32 + r] = 2 * i
        perm[qd * 32 + 16 + r] = 2 * i + 1
    return perm


def _rope_tables():
    """cos_dup/sin_signed [128, S]: per-partition rope tables matching the
    de-interleaved layout (pattern repeats every 64 partitions)."""
    inv_freq = 1.0 / (THETA ** (np.arange(0, HD, 2, dtype=np.float64) / HD))  # [32]
    pos = np.arange(S, dtype=np.float64)
    ang = pos[None, :] * inv_freq[:, None]  # [32, S]
    cos = np.cos(ang)
    sin = np.sin(ang)
    cos_dup = np.zeros((128, S), dtype=np.float32)
    sin_signed = np.zeros((128, S), dtype=np.float32)
    for p in range(128):
        d = p % HD
        qd, r0 = divmod(d, 32)
        if r0 < 16:
            i = qd * 16 + r0
            cos_dup[p] = cos[i]
            sin_signed[p] = -sin[i]
        else:
            i = qd * 16 + (r0 - 16)
            cos_dup[p] = cos[i]
            sin_signed[p] = sin[i]
    return cos_dup, sin_signed


def _build_program():
    import concourse.bass as bass
    from concourse import bacc, mybir
    import concourse.tile as tile

    f32 = mybir.dt.float32
    f32r = mybir.dt.float32r
    bf16 = mybir.dt.bfloat16
    ADD = mybir.AluOpType.add
    MULT = mybir.AluOpType.mult
    EXP = mybir.ActivationFunctionType.Exp
    SWAP16 = [(j + 16) % 32 for j in range(32)]
    NDK = D // 128  # contraction chunks for projections

    nc = bacc.Bacc("TRN2", target_bir_lowering=False, debug=False)
    xT = nc.dram_tensor("xT", [D, S], bf16, kind="ExternalInput").ap()
    wq = nc.dram_tensor("wq", [D, DC], bf16, kind="ExternalInput").ap()
    wk = nc.dram_tensor("wk", [D, DC], bf16, kind="ExternalInput").ap()
    wv = nc.dram_tensor("wv", [D, DC], bf16, kind="ExternalInput").ap()
    wo = nc.dram_tensor("wo", [DC, D], bf16, kind="ExternalInput").ap()
    cosd = nc.dram_tensor("cosd", [128, S], bf16, kind="ExternalInput").ap()
    sind = nc.dram_tensor("sind", [128, S], bf16, kind="ExternalInput").ap()
    tri = nc.dram_tensor("tri", [KC, KC], f32, kind="ExternalInput").ap()
    ones1 = nc.dram_tensor("ones1", [1, HD], f32r, kind="ExternalInput").ap()
    outT = nc.dram_tensor("outT", [D, S], f32, kind="ExternalOutput").ap()

    with tile.TileContext(nc) as tc:
        with tc.tile_pool(name="const", bufs=1) as const, \
             tc.tile_pool(name="tmp2", bufs=3) as tmp2, \
             tc.tile_pool(name="probs", bufs=8) as probs_pool, \
             tc.tile_pool(name="rsb", bufs=3) as rsb_pool, \
             tc.tile_pool(name="sums", bufs=2) as sums_pool, \
             tc.tile_pool(name="stage", bufs=4) as stage_pool, \
             tc.tile_pool(name="psb", bufs=2, space="PSUM") as psb, \
             tc.tile_pool(name="pss", bufs=3, space="PSUM") as pss, \
             tc.tile_pool(name="pso", bufs=3, space="PSUM") as pso:
            cos_sb = const.tile([128, S], bf16)
            sin_sb = const.tile([128, S], bf16)
            tri_sb = const.tile([KC, KC], f32)
            ones1_sb = const.tile([1, HD], f32r)
            wo_sb = const.tile([128, GQ, D], bf16)
            qT_sb = const.tile([128, GQ, S], bf16)
            kT_sb = const.tile([128, GQ, S], bf16)
            vaug_sb = const.tile([128, NKC, HPC * (HD + 1)], bf16)
            oT_sb = const.tile([128, GQ, S], bf16)
            xT_sb = [const.tile([128, S], bf16, name=f"x{kc}") for kc in range(NDK)]
            wq_sb = [const.tile([128, DC], bf16, name=f"wq{kc}") for kc in range(NDK)]
            wk_sb = [const.tile([128, DC], bf16, name=f"wk{kc}") for kc in range(NDK)]
            wv_sb = [const.tile([128, DC], bf16, name=f"wv{kc}") for kc in range(NDK)]

            # input DMA spread over three queues so x/w land in ~6 us
            nc.sync.dma_start(tri_sb, tri)
            nc.sync.dma_start(ones1_sb, ones1)
            for kc in range(NDK):
                nc.sync.dma_start(wq_sb[kc], wq[kc * 128:(kc + 1) * 128, :])
                nc.sync.dma_start(wk_sb[kc], wk[kc * 128:(kc + 1) * 128, :])
            nc.gpsimd.dma_start(cos_sb, cosd)
            nc.gpsimd.dma_start(sin_sb, sind)
            nc.gpsimd.memset(vaug_sb[:, :, HD::(HD + 1)], 1.0)
            for kc in range(NDK):
                q3 = [nc.scalar, nc.gpsimd, nc.sync][kc % 3]
                q3.dma_start(xT_sb[kc], xT[kc * 128:(kc + 1) * 128, :])
            for kc in range(NDK):
                nc.scalar.dma_start(wv_sb[kc], wv[kc * 128:(kc + 1) * 128, :])
            nc.gpsimd.dma_start(wo_sb, wo.rearrange("(o p) n -> p o n", p=128))

            dmaeng = [nc.sync, nc.gpsimd, nc.scalar]
            pending_op = []  # deferred out-projection work (one qt behind)

            def rope(ps, dst, q0):
                shuf = tmp2.tile([128, QT], f32, tag="shuf")
                nc.vector.stream_shuffle(shuf, ps, SWAP16)
                m1 = tmp2.tile([128, QT], f32, tag="m1")
                nc.vector.tensor_tensor(m1, ps, cos_sb[:, q0:q0 + QT], MULT)
                m2 = tmp2.tile([128, QT], f32, tag="m2")
                nc.vector.tensor_tensor(m2, shuf, sin_sb[:, q0:q0 + QT], MULT)
                nc.vector.tensor_tensor(dst, m1, m2, ADD)

            def emit_out_proj(qt):
                q0 = qt * QT
                for ec in range(D // 128):
                    ps = psb.tile([128, QT], f32, tag="b", name="op")
                    for g in range(GQ):
                        nc.tensor.matmul(
                            ps, (wo_sb[:, g, ec * 128:(ec + 1) * 128]),
                            (oT_sb[:, g, q0:q0 + QT]),
                            start=(g == 0), stop=(g == GQ - 1))
                    ob = stage_pool.tile([128, QT], f32, tag="ob")
                    nc.vector.tensor_copy(out=ob, in_=ps)
                    dmaeng[ec % 3].dma_start(
                        outT[ec * 128:(ec + 1) * 128, q0:q0 + QT], ob)

            for qt in range(NQT):
                q0 = qt * QT
                nkc = (q0 + QT) // KC
                # ---- q/k projection + rope for this q-tile ----
                for g in range(GQ):
                    ps_q = psb.tile([128, QT], f32, tag="b", name="q")
                    for kc in range(NDK):
                        st = dict(start=(kc == 0), stop=(kc == NDK - 1))
                        nc.tensor.matmul(
                            ps_q, (wq_sb[kc][:, g * 128:(g + 1) * 128]),
                            (xT_sb[kc][:, q0:q0 + QT]), **st)
                    ps_k = psb.tile([128, QT], f32, tag="b", name="k")
                    for kc in range(NDK):
                        st = dict(start=(kc == 0), stop=(kc == NDK - 1))
                        nc.tensor.matmul(
                            ps_k, (wk_sb[kc][:, g * 128:(g + 1) * 128]),
                            (xT_sb[kc][:, q0:q0 + QT]), **st)
                    rope(ps_q, qT_sb[:, g, q0:q0 + QT], q0)
                    rope(ps_k, kT_sb[:, g, q0:q0 + QT], q0)
                # ---- v projection for this tile's four 128-row chunks ----
                for rp in range(2):
                    ps_v = psb.tile([128, 2 * DC], f32, tag="b", name="v")
                    for half in range(2):
                        rc = 4 * qt + 2 * rp + half
                        for kc in range(NDK):
                            st = dict(start=(kc == 0), stop=(kc == NDK - 1))
                            nc.tensor.matmul(
                                ps_v[:, half * DC:(half + 1) * DC],
                                (xT_sb[kc][:, rc * 128:(rc + 1) * 128]),
                                (wv_sb[kc]), **st)
                    for half in range(2):
                        rc = 4 * qt + 2 * rp + half
                        for h in range(HPC):
                            nc.vector.tensor_copy(
                                out=vaug_sb[:, rc, h * (HD + 1):h * (HD + 1) + HD],
                                in_=ps_v[:, half * DC + h * HD:half * DC + (h + 1) * HD])

                # ---- causal flash attention for this q-tile ----
                for g in range(GQ):
                    ps_o = [pso.tile([HD + 1, QT], f32, tag="o",
                                     name=f"o{g}{a}") for a in range(2)]
                    pend = []  # (probs pair, kc, qlo) awaiting AV matmul

                    def emit_av(item, last, g=g, ps_o=ps_o):
                        pr, kc_, qlo_ = item
                        for a in range(2):
                            h = 2 * g + a
                            nc.tensor.matmul(
                                ps_o[a][:, qlo_:QT],
                                (vaug_sb[:, kc_, h * (HD + 1):(h + 1) * (HD + 1)]),
                                (pr[a][:, qlo_:QT]),
                                start=(kc_ == 0), stop=last)

                    for kc in range(nkc):
                        k0 = kc * KC
                        qlo = max(0, k0 - q0)
                        ps_s = [pss.tile([128, QT], f32, tag="s",
                                         name=f"s{a}") for a in range(2)]
                        for a in range(2):
                            nc.tensor.matmul(
                                ps_s[a][:, qlo:QT],
                                (kT_sb[a * HD:(a + 1) * HD, g, k0:k0 + KC]),
                                (qT_sb[a * HD:(a + 1) * HD, g, q0 + qlo:q0 + QT]),
                                start=True, stop=True)
                        if k0 >= q0:
                            for a in range(2):
                                nc.vector.tensor_tensor(
                                    ps_s[a][:, qlo:qlo + KC],
                                    ps_s[a][:, qlo:qlo + KC], tri_sb, ADD)
                        pr = [probs_pool.tile([128, QT], bf16, tag="p",
                                              name=f"p{a}") for a in range(2)]
                        for a in range(2):
                            nc.scalar.activation(
                                pr[a][:, qlo:QT], ps_s[a][:, qlo:QT], EXP)
                        pend.append((pr, kc, qlo))
                        if len(pend) > 2:
                            emit_av(pend.pop(0), last=False)
                    while pend:
                        emit_av(pend.pop(0), last=(len(pend) == 0))

                    # ---- on-chip normalization for this (qt, g) ----
                    sums_t = sums_pool.tile([1, 2, QT], f32r, tag="sm")
                    for a in range(2):
                        nc.vector.tensor_copy(
                            out=sums_t[0:1, a, :], in_=ps_o[a][HD:HD + 1, :])
                    for a in range(2):
                        ps_bc = pso.tile([HD + 1, QT], f32, tag="o", name="bc")
                        nc.tensor.matmul(
                            ps_bc[0:HD, :], ones1_sb, sums_t[0:1, a, :],
                            start=True, stop=True)
                        r_a = rsb_pool.tile([HD, QT], f32, tag="r")
                        nc.vector.reciprocal_approx_fast(out=r_a, in_=ps_bc[0:HD, :])
                        nc.vector.tensor_tensor(
                            oT_sb[a * HD:(a + 1) * HD, g, q0:q0 + QT],
                            ps_o[a][0:HD, :], r_a, MULT)

                # ---- output projection, one q-tile behind ----
                if qt > 0:
                    emit_out_proj(qt - 1)
            emit_out_proj(NQT - 1)

    nc.finalize()
    return nc


def kernel(x, wq, wk, wv, wo):
    from concourse import bass_utils

    if os.environ.get("BASS_TRACE"):
        _install_axon_ntff_hook()

    x = np.asarray(x, dtype=np.float32)
    wq = np.asarray(wq, dtype=np.float32)
    wk = np.asarray(wk, dtype=np.float32)
    wv = np.asarray(wv, dtype=np.float32)
    wo = np.asarray(wo, dtype=np.float32)

    # Host prep: weight slicing + rope column permutation + tables.
    perm_l = _rope_perm_local()
    perm = np.concatenate([h * HD + perm_l for h in range(NH)])  # [D]
    scale = 1.0 / np.sqrt(HD)
    wq_p = (wq[:, perm] * scale).astype(BF16)
    wk_p = wk[:, perm].astype(BF16)
    wv_b = wv.astype(BF16)
    wo_b = wo.astype(BF16)
    cos_dup, sin_signed = _rope_tables()
    kl = np.arange(KC)[:, None]
    ql = np.arange(KC)[None, :]
    tri = np.where(ql >= kl, 0.0, MASKVAL).astype(np.float32)

    xTs = [np.ascontiguousarray(x[b].T).astype(BF16) for b in range(B)]
    cos_b = cos_dup.astype(BF16)
    sin_b = sin_signed.astype(BF16)
    ones1 = np.ones((1, HD), np.float32)

    in_maps = []
    for i in range(NCORES):
        b, g = divmod(i, HPC)
        cs = slice(g * DC, (g + 1) * DC)
        in_maps.append({
            "xT": xTs[b],
            "wq": np.ascontiguousarray(wq_p[:, cs]),
            "wk": np.ascontiguousarray(wk_p[:, cs]),
            "wv": np.ascontiguousarray(wv_b[:, cs]),
            "wo": np.ascontiguousarray(wo_b[cs, :]),
            "cosd": cos_b,
            "sind": sin_b,
            "tri": tri,
            "ones1": ones1,
        })

    if "nc" not in _CACHE:
        _CACHE["nc"] = _build_program()
    nc = _CACHE["nc"]

    res = bass_utils.run_bass_kernel_spmd(nc, in_maps, core_ids=list(range(NCORES)))
    _CACHE["last_exec_time_ns"] = res.exec_time_ns
    _CACHE["last_res"] = res

    out = np.empty((B, S, D), dtype=np.float32)
    for b in range(B):
        acc = res.results[b * HPC]["outT"].copy()
        for g in range(1, HPC):
            acc += res.results[b * HPC + g]["outT"]
        out[b] = acc.T
    return out
